# revision 1
# baseline (speedup 1.0000x reference)
"""Multi-head attention (B=4,N=2048,C=768,H=12) on 8 trn2 NeuronCores.

Sharding: data-parallel over B (4 batches x 2 cores each), tensor-parallel
over heads (6 heads per core). Each core:
  - QKV projection for its 6 heads (f32r matmuls, fp32 accumulate)
  - transposed scores st[kv, q] = K_h^T-layout matmul, contraction D=64,
    two heads row-packed into PE partitions 0-63 / 64-127
  - exp on ScalarE (scale fused), bf16 output
  - attn@V with a ones-column appended to V (M=65) so the softmax
    denominator falls out of the same matmul
  - normalize on DVE (gpsimd partition_broadcast of the reciprocal row)
  - output projection (bf16, contraction 128 per head-pair; odd head
    DMA-shifted to partitions 64-127) -> per-pair partial y to DRAM
Host sums the six partials per batch (3 pairs x 2 cores) and adds the bias.
"""

import sys

import numpy as np
import ml_dtypes

_REPO = "/opt/trn_rl_repo"
if _REPO not in sys.path:
    sys.path.insert(0, _REPO)

import concourse.bacc as bacc
import concourse.mybir as mybir
import concourse.tile as tile
from concourse.bass_utils import run_bass_kernel_spmd

B, N, C, H, D = 4, 2048, 768, 12, 64
HL = H // 2          # heads per core
SCALE = D ** -0.5
NCORES = 8
KT_C = C // 128      # 6 contraction tiles over C
MT_QK = 2 * HL // 2  # 6 output tiles for q+k (3 Q pairs, 3 K pairs)
QS = N // 512        # 4 query strips
KVT = N // 128       # 16 kv tiles

F32 = mybir.dt.float32
F32R = mybir.dt.float32r
BF16 = mybir.dt.bfloat16
EXP = mybir.ActivationFunctionType.Exp

_CACHE = {}


def _build():
    nc = bacc.Bacc("TRN2", target_bir_lowering=False, debug=False,
                   num_devices=NCORES)
    xT = nc.dram_tensor("xT", [C, N], F32R, kind="ExternalInput").ap()
    wqkT = nc.dram_tensor("wqkT", [C, 2 * HL * D], F32R, kind="ExternalInput").ap()
    wvT = nc.dram_tensor("wvT", [C, HL * D], F32R, kind="ExternalInput").ap()
    wpT = nc.dram_tensor("wpT", [HL * D, C], BF16, kind="ExternalInput").ap()
    y = nc.dram_tensor("y", [HL // 2, N, C], F32, kind="ExternalOutput").ap()

    with tile.TileContext(nc) as tc:
        with (
            tc.tile_pool(name="singles", bufs=1) as singles,
            tc.tile_pool(name="ps_a", bufs=2, space="PSUM") as ps_a,
            tc.tile_pool(name="ps_st", bufs=2, space="PSUM") as ps_st,
            tc.tile_pool(name="ps_out", bufs=2, space="PSUM") as ps_out,
            tc.tile_pool(name="est", bufs=6) as est_p,
            tc.tile_pool(name="rec", bufs=4) as rec_p,
            tc.tile_pool(name="rb", bufs=4) as rb_p,
            tc.tile_pool(name="ysb", bufs=5) as ysb_p,
        ):
            xT_sb = singles.tile([128, KT_C, N], F32R)
            wqk_sb = singles.tile([128, KT_C, 2 * HL * D], F32R)
            wv_sb = singles.tile([128, KT_C, HL * D], F32R)
            wp_sb = singles.tile([128, HL // 2, C], BF16)
            qk_sb = singles.tile([128, MT_QK, N], F32R)
            v_sb = singles.tile([128, KVT, HL, D + 1], BF16)
            # attention output in proj-ready pair layout: [128, pair, N]
            # (even head -> partitions 0-63 via DVE, odd head staged at 0-63
            # then DMA-shifted to partitions 64-127)
            attn_sb = singles.tile([128, HL // 2, N], BF16)

            for kt in range(KT_C):
                for hf in range(2):
                    hsl = slice(hf * (N // 2), (hf + 1) * (N // 2))
                    nc.sync.dma_start(xT_sb[:, kt, hsl],
                                      xT[kt * 128:(kt + 1) * 128, hsl])
                nc.sync.dma_start(wqk_sb[:, kt, :], wqkT[kt * 128:(kt + 1) * 128, :])
            for kt in range(KT_C):
                nc.sync.dma_start(wv_sb[:, kt, :], wvT[kt * 128:(kt + 1) * 128, :])
            for p in range(HL // 2):
                nc.sync.dma_start(wp_sb[:, p, :], wpT[p * 128:(p + 1) * 128, :])
            nc.vector.memset(v_sb[:, :, :, D:D + 1], 1.0)

            # warm the ACT exp table set during the DMA fill so the ~2.7us
            # ACT_TABLE_LOAD is off the first real exp's critical path
            warm_in = rec_p.tile([1, 2], F32, tag="warm")
            warm_out = rec_p.tile([1, 2], BF16, tag="warmo")
            nc.vector.memset(warm_in, 0.0)
            nc.scalar.activation(warm_out, warm_in, EXP, scale=SCALE)

            # PE is otherwise idle until the first xT tiles land (~6us);
            # dependency-free dummy matmuls fill that window and keep the
            # HAM clock-gate warm so the first real matmuls run at 2.4GHz
            nc.vector.memset(attn_sb[:, 0, 0:640], 0.0)
            for _ in range(16):
                warm_ps = ps_out.tile([128, 512], F32, tag="out")
                nc.tensor.matmul(warm_ps, lhsT=attn_sb[:, 0, 0:128],
                                 rhs=attn_sb[:, 0, 128:640])

            def emit_qk_tile(t, pools=None):
                for qs in range(QS):
                    pool, tag = (pools[qs % len(pools)] if pools
                                 else (ps_a, "ps_a"))
                    ps = pool.tile([128, 512], F32, tag=tag)
                    for kt in range(KT_C):
                        nc.tensor.matmul(
                            ps,
                            lhsT=wqk_sb[:, kt, t * 128:(t + 1) * 128],
                            rhs=xT_sb[:, kt, qs * 512:(qs + 1) * 512],
                            start=(kt == 0), stop=(kt == KT_C - 1),
                        )
                    nc.vector.tensor_copy(qk_sb[:, t, qs * 512:(qs + 1) * 512], ps)

            def emit_v(mts, pools=None):
                for mt in mts:
                    pool, tag = (pools[mt % len(pools)] if pools
                                 else (ps_a, "ps_a"))
                    ps = pool.tile([128, HL * D], F32, tag=tag)
                    for kt in range(KT_C):
                        nc.tensor.matmul(
                            ps,
                            lhsT=xT_sb[:, kt, mt * 128:(mt + 1) * 128],
                            rhs=wv_sb[:, kt, :],
                            start=(kt == 0), stop=(kt == KT_C - 1),
                        )
                    nc.vector.tensor_copy(
                        v_sb[:, mt, :, 0:D],
                        ps.rearrange("p (h d) -> p h d", h=HL),
                    )

            def emit_attention_pair(pr):
                tq, tk = pr, HL // 2 + pr
                for qs in range(QS):
                    qsl = slice(qs * 512, (qs + 1) * 512)
                    out_a = ps_out.tile([128, 512], F32, tag="out")
                    out_b = ps_out.tile([128, 512], F32, tag="out")
                    outs = [out_a, out_b]
                    for kt in range(KVT):
                        # both heads' scores into one 2-bank tile, one exp
                        st = ps_st.tile([128, 2, 512], F32, tag="st")
                        for half in range(2):
                            p0, p1 = half * 64, (half + 1) * 64
                            nc.tensor.matmul(
                                st[:, half, :],
                                lhsT=qk_sb[p0:p1, tk, kt * 128:(kt + 1) * 128],
                                rhs=qk_sb[p0:p1, tq, qsl],
                            )
                        est = est_p.tile([128, 2, 512], BF16, tag="est")
                        nc.scalar.activation(est, st, EXP, scale=SCALE)
                        for half in range(2):
                            h = 2 * pr + half
                            nc.tensor.matmul(
                                outs[half][0:D + 1, :],
                                lhsT=v_sb[:, kt, h, :],
                                rhs=est[:, half, :],
                                start=(kt == 0), stop=(kt == KVT - 1),
                            )
                    for half in range(2):
                        out_ps = outs[half]
                        # stage to SBUF right away so the PSUM bank frees
                        # before the (slow) normalize chain runs
                        ostg = rec_p.tile([65, 512], F32, tag="ostg")
                        nc.vector.tensor_copy(ostg, out_ps[0:D + 1, :])
                        rec = rec_p.tile([1, 512], F32, tag="rec")
                        nc.vector.reciprocal(rec, ostg[D:D + 1, :])
                        rb = rb_p.tile([64, 512], F32, tag="rb")
                        nc.gpsimd.partition_broadcast(rb, rec)
                        if half == 0:
                            nc.vector.tensor_mul(
                                attn_sb[0:64, pr, qsl], ostg[0:D, :], rb)
                        else:
                            stg = rb_p.tile([64, 512], BF16, tag="astg")
                            nc.vector.tensor_mul(stg, ostg[0:D, :], rb)
                            # shift odd head into partitions 64-127
                            nc.sync.dma_start(attn_sb[64:128, pr, qsl], stg)

            # per-pair proj partials straight to DRAM (summed on host) so
            # proj overlaps with the next pair's attention
            def emit_proj_pair(pr, pools=None):
                for mt in range(KVT):
                    ysb = ysb_p.tile([128, 2, 384], F32, tag="ysb")
                    for ns in range(2):
                        pool, tag = (pools[(2 * mt + ns) % len(pools)] if pools
                                     else (ps_a, "ps_a"))
                        yp = pool.tile([128, 384], F32, tag=tag)
                        nc.tensor.matmul(
                            yp,
                            lhsT=attn_sb[:, pr, mt * 128:(mt + 1) * 128],
                            rhs=wp_sb[:, pr, ns * 384:(ns + 1) * 384],
                        )
                        nc.vector.tensor_copy(ysb[:, ns, :], yp)
                    nc.sync.dma_start(
                        y[pr, mt * 128:(mt + 1) * 128, :],
                        ysb.rearrange("p a b -> p (a b)"))

            # Emission order = scheduler priority. Attention (ACT-bound)
            # leads; qk tiles for the NEXT pair and proj for the current
            # pair are emitted after it so the in-order PE slots them into
            # the ~190ns/iter gaps where it waits on ScalarE's exp.
            # pre-attention phase may borrow the (idle) attention PSUM
            # slots so more accumulation groups overlap the xT DMA fill
            fill_pools = [(ps_a, "ps_a"), (ps_st, "st"), (ps_out, "out")]
            emit_qk_tile(0, fill_pools)
            emit_qk_tile(HL // 2, fill_pools)
            emit_v(range(KVT), fill_pools)
            for pr in range(HL // 2):
                emit_attention_pair(pr)
                if pr + 1 < HL // 2:
                    emit_qk_tile(pr + 1)
                    emit_qk_tile(HL // 2 + pr + 1)
                emit_proj_pair(pr)

    nc.compile()
    return nc


def _get_nc():
    if "nc" not in _CACHE:
        _CACHE["nc"] = _build()
    return _CACHE["nc"]


def _prep_inputs(x, w_qkv, w_proj):
    """Per-core input dicts. Core c: batch c//2, head-half c%2."""
    wq, wk, wv = w_qkv[0:C], w_qkv[C:2 * C], w_qkv[2 * C:3 * C]
    in_maps = []
    for core in range(NCORES):
        b, p = divmod(core, 2)
        heads = [p * HL + j for j in range(HL)]
        qk_rows = np.concatenate(
            [wq[h * D:(h + 1) * D] for h in heads]
            + [wk[h * D:(h + 1) * D] for h in heads], axis=0)   # [768, C]
        v_rows = np.concatenate(
            [wv[h * D:(h + 1) * D] for h in heads], axis=0)     # [384, C]
        p_cols = np.concatenate(
            [w_proj[:, h * D:(h + 1) * D] for h in heads], axis=1)  # [C, 384]
        in_maps.append({
            "xT": np.ascontiguousarray(x[b].T, dtype=np.float32),
            "wqkT": np.ascontiguousarray(qk_rows.T, dtype=np.float32),
            "wvT": np.ascontiguousarray(v_rows.T, dtype=np.float32),
            "wpT": np.ascontiguousarray(p_cols.T).astype(ml_dtypes.bfloat16),
        })
    return in_maps


def kernel(x, w_qkv, w_proj, b_proj, _trace=False):
    x = np.asarray(x, dtype=np.float32)
    w_qkv = np.asarray(w_qkv, dtype=np.float32)
    w_proj = np.asarray(w_proj, dtype=np.float32)
    b_proj = np.asarray(b_proj, dtype=np.float32)

    nc = _get_nc()
    in_maps = _prep_inputs(x, w_qkv, w_proj)
    # retry: transient NRT_EXEC_UNIT_UNRECOVERABLE has been observed once
    # on a first attempt and succeeded immediately on retry
    last_exc = None
    for _attempt in range(3):
        try:
            res = run_bass_kernel_spmd(nc, in_maps,
                                       core_ids=list(range(NCORES)),
                                       trace=_trace)
            break
        except Exception as e:
            last_exc = e
    else:
        raise last_exc
    _CACHE["last_results"] = res

    out = np.empty((B, N, C), dtype=np.float32)
    for b in range(B):
        out[b] = (res.results[2 * b]["y"].sum(0)
                  + res.results[2 * b + 1]["y"].sum(0) + b_proj)
    return out

